# revision 1
# baseline (speedup 1.0000x reference)
"""AttentiveTransformer (linear -> ghost BN -> prior mask -> sparsemax) on 8 TRN2 cores.

Layout: batch rows on partitions, G=2048 on free axis. Each [128, 2048] tile is
exactly one ghost-BN chunk. Per core: batch shard of 8192 rows = 64 tiles.

Math per chunk c:
  f~  = f - colmean(f)                      (centers x since matmul is linear)
  x~  = f~ @ W.T                            (f32r matmuls, PE)
  var = sum_p(x~^2)/128                     (ones-matmul over squared output)
  a   = gamma * rsqrt(var + eps)
  z   = x~ * a * priors
  out = relu(z - tau(z)),  tau = max_r (cumsum(top16(z))_r - 1)/r
"""
import numpy as np
from contextlib import ExitStack

import concourse.bass as bass
import concourse.bacc as bacc
import concourse.tile as tile
from concourse import mybir
from concourse.bass_utils import run_bass_kernel_spmd
import concourse.bass_isa as bass_isa

F32 = mybir.dt.float32
F32R = mybir.dt.float32r
BN_EPS = 1e-5
NEG_BIG = -1.0e30

B_FULL, IN, G = 65536, 512, 2048
N_CORES = 8
P = 128
NT = G // 512          # 4 n-tiles of 512
KT = IN // 128         # 4 k-tiles of 128


def build(n_tiles, gamma_trivial, beta_zero):
    nc = bacc.Bacc()
    rows = n_tiles * P
    feat_d = nc.dram_tensor("feat", [rows, IN], F32, kind="ExternalInput")
    priors_d = nc.dram_tensor("priors", [rows, G], F32, kind="ExternalInput")
    w_d = nc.dram_tensor("w", [G, IN], F32, kind="ExternalInput")
    ident_d = nc.dram_tensor("ident", [P, P], F32, kind="ExternalInput")
    rinv_d = nc.dram_tensor("rinv", [P, 16], F32, kind="ExternalInput")
    gs_d = nc.dram_tensor("gs", [1, G], F32, kind="ExternalInput")
    ones_d = nc.dram_tensor("ones1", [P, 1], F32, kind="ExternalInput")
    onescol_d = nc.dram_tensor("onescol", [1, P], F32, kind="ExternalInput")
    negones_d = nc.dram_tensor("negones", [P, 1], F32, kind="ExternalInput")
    out_d = nc.dram_tensor("out", [rows, G], F32, kind="ExternalOutput")

    with tile.TileContext(nc) as tc, ExitStack() as ctx:
        singles = ctx.enter_context(tc.tile_pool(name="singles", bufs=1))
        fpool = ctx.enter_context(tc.tile_pool(name="fpool", bufs=3))
        ftpool = ctx.enter_context(tc.tile_pool(name="ftpool", bufs=4))
        xpool = ctx.enter_context(tc.tile_pool(name="xpool", bufs=10))
        sqpool = ctx.enter_context(tc.tile_pool(name="sqpool", bufs=6))
        ppool = ctx.enter_context(tc.tile_pool(name="ppool", bufs=3))
        zpool = ctx.enter_context(tc.tile_pool(name="zpool", bufs=4))
        zrpool = ctx.enter_context(tc.tile_pool(name="zrpool", bufs=2))
        smpool = ctx.enter_context(tc.tile_pool(name="smpool", bufs=8))
        arowpool = ctx.enter_context(tc.tile_pool(name="arowpool", bufs=6))
        areppool = ctx.enter_context(tc.tile_pool(name="areppool", bufs=8))
        wpool = ctx.enter_context(tc.tile_pool(name="wpool", bufs=2))
        ps_t = ctx.enter_context(tc.tile_pool(name="ps_t", bufs=1, space="PSUM"))
        ps_x = ctx.enter_context(tc.tile_pool(name="ps_x", bufs=4, space="PSUM"))
        ps_s = ctx.enter_context(tc.tile_pool(name="ps_s", bufs=2, space="PSUM"))
        ps_b = ctx.enter_context(tc.tile_pool(name="ps_b", bufs=1, space="PSUM"))

        # ---- constants ----
        ident = singles.tile([P, P], F32)
        nc.sync.dma_start(ident[:], ident_d[:])
        rinv = singles.tile([P, 16], F32)
        nc.sync.dma_start(rinv[:], rinv_d[:])
        gs = singles.tile([1, G], F32)
        nc.sync.dma_start(gs[:], gs_d[:])
        ones_f = singles.tile([P, 1], F32)
        nc.sync.dma_start(ones_f[:], ones_d[:])
        ones_r = singles.tile([P, 1], F32R)
        nc.scalar.copy(ones_r[:], ones_f[:])
        onescol_f = singles.tile([1, P], F32)
        nc.sync.dma_start(onescol_f[:], onescol_d[:])
        onescol_r = singles.tile([1, P], F32R)
        nc.scalar.copy(onescol_r[:], onescol_f[:])
        negones = singles.tile([P, 1], F32)
        nc.sync.dma_start(negones[:], negones_d[:])
        zeros16 = singles.tile([P, 16], F32)
        nc.vector.memset(zeros16[:], 0.0)
        eps_t = singles.tile([1, 1], F32)
        nc.vector.memset(eps_t[:], BN_EPS)

        # ---- W -> Wt (transposed, f32r) ----
        wt = singles.tile([P, KT, G], F32R)  # wt[i, k, g] = W[g, k*128+i]
        for gt in range(G // P):
            w_t = wpool.tile([P, IN], F32)
            nc.sync.dma_start(w_t[:], w_d[gt * P:(gt + 1) * P, :])
            for k in range(KT):
                pt = ps_t.tile([P, P], F32)
                nc.tensor.transpose(pt[:], w_t[:, k * P:(k + 1) * P], ident[:])
                nc.scalar.copy(wt[:, k, gt * P:(gt + 1) * P], pt[:])

        # ---- per-tile pipeline ----
        for c in range(n_tiles):
            f = fpool.tile([P, IN], F32)
            nc.sync.dma_start(f[:], feat_d[c * P:(c + 1) * P, :])
            ar = fpool.tile([P, IN], F32, tag="ar")
            nc.gpsimd.partition_all_reduce(ar[:], f[:], channels=P,
                                           reduce_op=bass_isa.ReduceOp.add)
            fc = fpool.tile([P, IN], F32, tag="fc")
            nc.vector.scalar_tensor_tensor(fc[:], in0=ar[:], scalar=-1.0 / P, in1=f[:],
                                           op0=mybir.AluOpType.mult,
                                           op1=mybir.AluOpType.add)
            fT = ftpool.tile([P, KT, P], F32R)
            for k in range(KT):
                pt = ps_t.tile([P, P], F32)
                nc.tensor.transpose(pt[:], fc[:, k * P:(k + 1) * P], ident[:])
                nc.scalar.copy(fT[:, k, :], pt[:])

            p_t = ppool.tile([P, G], F32)
            nc.sync.dma_start(p_t[:], priors_d[c * P:(c + 1) * P, :])

            z = zpool.tile([P, G], F32)
            for n in range(NT):
                px = ps_x.tile([P, 512], F32, tag="px")
                for k in range(KT):
                    nc.tensor.matmul(px[:], fT[:, k, :],
                                     wt[:, k, n * 512:(n + 1) * 512],
                                     start=(k == 0), stop=(k == KT - 1))
                xsq = sqpool.tile([P, 512], F32R)
                nc.scalar.activation(xsq[:], px[:], mybir.ActivationFunctionType.Square)
                # u = x~ * priors straight from PSUM (no x_sb copy)
                u = xpool.tile([P, 512], F32)
                nc.vector.tensor_tensor(u[:], px[:], p_t[:, n * 512:(n + 1) * 512],
                                        op=mybir.AluOpType.mult)
                vps = ps_s.tile([1, 512], F32)
                nc.tensor.matmul(vps[:], ones_r[:], xsq[:], start=True, stop=True)
                # srow = sqrt(var + eps) as f32r row; rank-1 PE broadcast; 1/x on DVE
                srow = arowpool.tile([1, 512], F32R, tag="arow")
                nc.scalar.activation(srow[:], vps[:], mybir.ActivationFunctionType.Sqrt,
                                     bias=eps_t[:])
                sps = ps_b.tile([P, 512], F32)
                nc.tensor.matmul(sps[:], onescol_r[:], srow[:], start=True, stop=True)
                arep = areppool.tile([P, 512], F32, tag="arep")
                nc.vector.reciprocal_approx_fast(arep[:], sps[:])
                if not gamma_trivial:
                    grow = arowpool.tile([1, 512], F32R, tag="grow")
                    nc.scalar.activation(grow[:], gs[:, n * 512:(n + 1) * 512],
                                         mybir.ActivationFunctionType.Copy)
                    gps = ps_b.tile([P, 512], F32)
                    nc.tensor.matmul(gps[:], onescol_r[:], grow[:], start=True, stop=True)
                    nc.vector.tensor_tensor(arep[:], arep[:], gps[:],
                                            op=mybir.AluOpType.mult)
                zn = z[:, n * 512:(n + 1) * 512]
                nc.vector.tensor_tensor(zn, u[:], arep[:], op=mybir.AluOpType.mult)

            # top-16 + tau
            m16 = smpool.tile([P, 16], F32)
            zr = zrpool.tile([P, G], F32)
            nc.vector.max(m16[:, 0:8], z[:])
            nc.vector.match_replace(zr[:], in_to_replace=m16[:, 0:8], in_values=z[:],
                                    imm_value=NEG_BIG)
            nc.vector.max(m16[:, 8:16], zr[:])
            cs = smpool.tile([P, 16], F32)
            nc.vector.tensor_tensor_scan(cs[:], m16[:], zeros16[:], 0.0,
                                         op0=mybir.AluOpType.add,
                                         op1=mybir.AluOpType.bypass)
            taur = smpool.tile([P, 16], F32)
            nc.vector.scalar_tensor_tensor(taur[:], in0=cs[:], scalar=-1.0, in1=rinv[:],
                                           op0=mybir.AluOpType.add,
                                           op1=mybir.AluOpType.mult)
            ntau = smpool.tile([P, 1], F32)
            nc.vector.tensor_reduce(ntau[:], taur[:], axis=mybir.AxisListType.X,
                                    op=mybir.AluOpType.max, negate=True)
            for n in range(NT):
                zn = z[:, n * 512:(n + 1) * 512]
                nc.scalar.activation(zn, zn, mybir.ActivationFunctionType.Relu,
                                     bias=ntau[:])
            nc.sync.dma_start(out_d[c * P:(c + 1) * P, :], z[:])

    nc.finalize()
    return nc


def _consts():
    ident = np.eye(P, dtype=np.float32)
    rinv = np.broadcast_to(1.0 / np.arange(1, 17, dtype=np.float32), (P, 16)).copy()
    ones1 = np.full((P, 1), 1.0 / P, dtype=np.float32)
    return ident, rinv, ones1


_CACHE = {}


def kernel(priors, processed_feat, W, gamma, beta):
    priors = np.ascontiguousarray(priors, dtype=np.float32)
    feat = np.ascontiguousarray(processed_feat, dtype=np.float32)
    W = np.ascontiguousarray(W, dtype=np.float32)
    gamma = np.asarray(gamma, dtype=np.float32)
    beta = np.asarray(beta, dtype=np.float32)

    B = feat.shape[0]
    n_cores = N_CORES
    shard = B // n_cores
    n_tiles = shard // P
    gamma_trivial = bool(np.all(gamma == 1.0))
    beta_zero = bool(np.all(beta == 0.0))
    assert beta_zero, "beta != 0 path not implemented"

    key = (n_tiles, gamma_trivial, beta_zero)
    if key not in _CACHE:
        _CACHE[key] = build(*key)
    nc = _CACHE[key]

    ident, rinv, ones1 = _consts()
    onescol = np.ones((1, P), dtype=np.float32)
    negones = np.full((P, 1), -1.0 / P, dtype=np.float32)
    gs = gamma.reshape(1, G)
    in_maps = []
    for i in range(n_cores):
        in_maps.append({
            "feat": feat[i * shard:(i + 1) * shard],
            "priors": priors[i * shard:(i + 1) * shard],
            "w": W,
            "ident": ident,
            "rinv": rinv,
            "gs": gs,
            "ones1": ones1,
            "onescol": onescol,
            "negones": negones,
        })
    res = run_bass_kernel_spmd(nc, in_maps, core_ids=list(range(n_cores)))
    return np.concatenate([r["out"] for r in res.results], axis=0)



# revision 13
# speedup vs baseline: 1.5239x; 1.5239x over previous
"""AttentiveTransformer (linear -> ghost BN -> prior mask -> sparsemax) on 8 TRN2 cores.

v2: fp16 end-to-end. Layout: batch rows on partitions, G=2048 on free axis.
Each [128, 2048] tile is one ghost-BN chunk; 64 tiles per core.

Per tile c:
  fT   = DMA-transposed f (xbar), fp16                  [128f, b]x4k
  nm   = -colmean(f) via 4 tiny PE matmuls (negsel)     [128f, 4k] psum
  fcT  = fT + nm (DVE tensor_scalar, 4x mode)
  x    = fcT.T @ Wt (16 fp16 matmuls, f32 psum)
  xs   = copy(x) fp16 (Act)         xsq = xs*xs (DVE)
  var  = selector-matmul colmean(xsq) -> [8,512] psum shared by tile pair
  a    = sqrt(1/var): DVE reciprocal + Act sqrt -> fp16 row
  rbc  = row broadcast by 2-stage DMA (gather row -> stride-0 bcast)
  ma   = priors * rbc (GPSIMD)      z = xs * ma (DVE 2x)
  tau  = top8 (DVE max8) cumsum trick;  out = Relu(z + (-tau)) on Act -> fp16
"""
import numpy as np
from contextlib import ExitStack

import concourse.bass as bass
import concourse.bacc as bacc
import concourse.tile as tile
from concourse import mybir
from concourse.bass_utils import run_bass_kernel_spmd

F32 = mybir.dt.float32
F16 = mybir.dt.float16

B_FULL, IN, G = 65536, 512, 2048
N_CORES = 8
P = 128
KT = IN // P           # 4 k-tiles of 128
NT = G // 512          # 4 n-tiles of 512


def build(n_tiles, gamma_trivial):
    nc = bacc.Bacc()
    rows = n_tiles * P
    f16_d = nc.dram_tensor("f16", [rows, IN], F16, kind="ExternalInput")
    p16_d = nc.dram_tensor("p16", [rows, G], F16, kind="ExternalInput")
    wt_d = nc.dram_tensor("wt", [P, KT * G], F16, kind="ExternalInput")
    selbig_d = nc.dram_tensor("selbig", [P, 64], F16, kind="ExternalInput")
    negsel_d = nc.dram_tensor("negsel", [P, 16], F16, kind="ExternalInput")
    rinv8_d = nc.dram_tensor("rinv8", [P, 8], F32, kind="ExternalInput")
    grow8_d = nc.dram_tensor("grow8", [8, 512], F16, kind="ExternalInput")
    out_d = nc.dram_tensor("out16", [rows, G], F16, kind="ExternalOutput")

    with tile.TileContext(nc) as tc, ExitStack() as ctx:
        singles = ctx.enter_context(tc.tile_pool(name="singles", bufs=1))
        ftpool = ctx.enter_context(tc.tile_pool(name="ftpool", bufs=2))
        f4pool = ctx.enter_context(tc.tile_pool(name="f4pool", bufs=2))
        fcpool = ctx.enter_context(tc.tile_pool(name="fcpool", bufs=3))
        ppool = ctx.enter_context(tc.tile_pool(name="ppool", bufs=5))
        xspool = ctx.enter_context(tc.tile_pool(name="xspool", bufs=5))
        xqpool = ctx.enter_context(tc.tile_pool(name="xqpool", bufs=3))
        vrpool = ctx.enter_context(tc.tile_pool(name="vrpool", bufs=2))
        arpool = ctx.enter_context(tc.tile_pool(name="arpool", bufs=2))
        rbpool = ctx.enter_context(tc.tile_pool(name="rbpool", bufs=3))
        mapool = ctx.enter_context(tc.tile_pool(name="mapool", bufs=3))
        zpool = ctx.enter_context(tc.tile_pool(name="zpool", bufs=3))
        smpool = ctx.enter_context(tc.tile_pool(name="smpool", bufs=4))
        opool = ctx.enter_context(tc.tile_pool(name="opool", bufs=3))
        adram = ctx.enter_context(tc.tile_pool(name="adram", bufs=2, space="DRAM"))
        ps_px = ctx.enter_context(tc.tile_pool(name="ps_px", bufs=6, space="PSUM"))
        ps_v = ctx.enter_context(tc.tile_pool(name="ps_v", bufs=1, space="PSUM"))
        ps_nm = ctx.enter_context(tc.tile_pool(name="ps_nm", bufs=1, space="PSUM"))

        # ---- constants ----
        wt_t = singles.tile([P, KT, G], F16)
        nc.sync.dma_start(wt_t[:], wt_d[:].rearrange("p (k g) -> p k g", k=KT))
        selbig = singles.tile([P, 8, 8], F16)
        nc.sync.dma_start(selbig[:], selbig_d[:].rearrange("p (a b) -> p a b", a=8))
        negsel = singles.tile([P, KT, 4], F16)
        nc.sync.dma_start(negsel[:], negsel_d[:].rearrange("p (a b) -> p a b", a=KT))
        rinv8 = singles.tile([P, 8], F32)
        nc.sync.dma_start(rinv8[:], rinv8_d[:])
        zeros8 = singles.tile([P, 8], F32)
        nc.vector.memset(zeros8[:], 0.0)
        if not gamma_trivial:
            grow8 = singles.tile([8, 512], F16)
            nc.sync.dma_start(grow8[:], grow8_d[:])

        AL = mybir.AluOpType

        def front(c, fTg, f4, vps8, first_of_pair):
            t16, t4 = c % 16, c % 4
            p16t = ppool.tile([P, G], F16)
            nc.sync.dma_start(p16t[:], p16_d[c * P:(c + 1) * P, :])
            # nm[:, k] = -colmean over batch of f feature block k
            nm = ps_nm.tile([P, KT], F32)
            for k in range(KT):
                nc.tensor.matmul(nm[:], f4[:, t4, k * P:(k + 1) * P],
                                 negsel[:, k, :], start=(k == 0), stop=(k == KT - 1))
            fcT = fcpool.tile([P, KT, P], F16)
            for k in range(KT):
                nc.vector.tensor_scalar(fcT[:, k, :],
                                        fTg[:, k, t16 * P:(t16 + 1) * P],
                                        nm[:, k:k + 1], None, op0=AL.add)
            pxq = [ps_px.tile([P, 512], F32, tag="px", name=f"px{n}")
                   for n in range(NT)]
            for k in range(KT):
                for n in range(NT):
                    nc.tensor.matmul(pxq[n][:], fcT[:, k, :],
                                     wt_t[:, k, n * 512:(n + 1) * 512],
                                     start=(k == 0), stop=(k == KT - 1))
            xs = xspool.tile([P, G], F16)
            for n in range(NT):
                nc.scalar.activation(xs[:, n * 512:(n + 1) * 512], pxq[n][:],
                                     mybir.ActivationFunctionType.Copy)
            xsq = xqpool.tile([P, G], F16)
            nc.vector.tensor_tensor(xsq[:], xs[:], xs[:], op=AL.mult)
            i = 0 if first_of_pair else 1
            for n in range(NT):
                nc.tensor.matmul(vps8[:], selbig[:, 4 * i + n, :],
                                 xsq[:, n * 512:(n + 1) * 512],
                                 start=(i == 0 and n == 0),
                                 stop=(i == 1 and n == NT - 1))
            return p16t, xs

        def finalize(vps8):
            vrec = vrpool.tile([8, 512], F32)
            nc.vector.reciprocal_approx_fast(vrec[:], vps8[:])
            arow = arpool.tile([8, 512], F16)
            nc.scalar.activation(arow[:], vrec[:], mybir.ActivationFunctionType.Sqrt)
            if not gamma_trivial:
                nc.vector.tensor_tensor(arow[:], arow[:], grow8[:], op=AL.mult)
            ascr = adram.tile([1, 4096], F16)
            nc.sync.dma_start(ascr[:], arow[:])
            rbc = rbpool.tile([P, 2, G], F16)
            nc.sync.dma_start(rbc[:].rearrange("p a b -> p (a b)"),
                              ascr[:].to_broadcast([P, 4096]))
            return rbc

        def back(c, i, p16t, xs, rbc):
            ma = mapool.tile([P, G], F16)
            nc.gpsimd.tensor_tensor(ma[:], p16t[:], rbc[:, i, :], op=AL.mult)
            z = zpool.tile([P, G], F16)
            nc.vector.tensor_tensor(z[:], xs[:], ma[:], op=AL.mult)
            m8 = smpool.tile([P, 8], F16, tag="m8")
            nc.vector.max(m8[:], z[:])
            cs = smpool.tile([P, 8], F32, tag="cs")
            nc.vector.tensor_tensor_scan(cs[:], m8[:], zeros8[:], 0.0,
                                         op0=AL.add, op1=AL.bypass)
            taur = smpool.tile([P, 8], F32, tag="taur")
            nc.vector.scalar_tensor_tensor(taur[:], in0=cs[:], scalar=-1.0,
                                           in1=rinv8[:], op0=AL.add, op1=AL.mult)
            ntau = smpool.tile([P, 1], F32, tag="ntau")
            nc.vector.tensor_reduce(ntau[:], taur[:], axis=mybir.AxisListType.X,
                                    op=AL.max, negate=True)
            o16 = opool.tile([P, G], F16)
            nc.scalar.activation(o16[:], z[:], mybir.ActivationFunctionType.Relu,
                                 bias=ntau[:])
            nc.sync.dma_start(out_d[c * P:(c + 1) * P, :], o16[:])

        prev = None
        fTg = f4 = None
        for pr in range(n_tiles // 2):
            c0 = 2 * pr
            if c0 % 16 == 0:
                fTg = ftpool.tile([P, KT, 2048], F16)
                g0 = c0 * P
                for k in range(KT):
                    nc.sync.dma_start_transpose(
                        fTg[:, k, :], f16_d[g0:g0 + 2048, k * P:(k + 1) * P])
            if c0 % 4 == 0:
                f4 = f4pool.tile([P, 4, IN], F16)
                nc.sync.dma_start(
                    f4[:], f16_d[c0 * P:(c0 + 4) * P, :].rearrange(
                        "(t p) k -> p t k", p=P))
            vps8 = ps_v.tile([8, 512], F32)
            pA = front(c0, fTg, f4, vps8, True)
            pB = front(c0 + 1, fTg, f4, vps8, False)
            if prev is not None:
                (a0, a1), (xa, xb), rbc_p, cp = prev
                back(cp, 0, a0, xa, rbc_p)
                back(cp + 1, 1, a1, xb, rbc_p)
            rbc = finalize(vps8)
            prev = ((pA[0], pB[0]), (pA[1], pB[1]), rbc, c0)
        (a0, a1), (xa, xb), rbc_p, cp = prev
        back(cp, 0, a0, xa, rbc_p)
        back(cp + 1, 1, a1, xb, rbc_p)

    nc.finalize()
    return nc


_CACHE = {}


def _consts():
    selbig = np.zeros((P, 8, 8), np.float16)
    for i in range(8):
        selbig[:, i, i] = 1.0 / 128
    negsel = np.zeros((P, KT, 4), np.float16)
    for k in range(KT):
        negsel[:, k, k] = -1.0 / 128
    rinv8 = np.broadcast_to(1.0 / np.arange(1, 9, dtype=np.float32), (P, 8)).copy()
    return selbig.reshape(P, 64), negsel.reshape(P, 16), rinv8


def kernel(priors, processed_feat, W, gamma, beta):
    feat = np.ascontiguousarray(processed_feat, dtype=np.float32)
    priors = np.ascontiguousarray(priors, dtype=np.float32)
    W = np.ascontiguousarray(W, dtype=np.float32)
    gamma = np.asarray(gamma, dtype=np.float32)
    beta = np.asarray(beta, dtype=np.float32)
    assert np.all(beta == 0.0), "beta != 0 path not implemented"
    gamma_trivial = bool(np.all(gamma == 1.0))

    B = feat.shape[0]
    shard = B // N_CORES
    n_tiles = shard // P

    f16 = feat.astype(np.float16)
    p16 = priors.astype(np.float16)
    # wt[p, k, g] = W[g, k*128+p]
    wt = np.ascontiguousarray(
        W.T.astype(np.float16).reshape(KT, P, G).transpose(1, 0, 2)
    ).reshape(P, KT * G)
    selbig, negsel, rinv8 = _consts()
    # grow8[4i+n, j] = gamma[n*512+j] for i in {0,1}
    gr = gamma.astype(np.float16).reshape(4, 512)
    grow8 = np.concatenate([gr, gr], axis=0)

    key = (n_tiles, gamma_trivial)
    if key not in _CACHE:
        _CACHE[key] = build(*key)
    nc = _CACHE[key]

    in_maps = []
    for i in range(N_CORES):
        in_maps.append({
            "f16": f16[i * shard:(i + 1) * shard],
            "p16": p16[i * shard:(i + 1) * shard],
            "wt": wt,
            "selbig": selbig,
            "negsel": negsel,
            "rinv8": rinv8,
            "grow8": grow8,
        })
    res = run_bass_kernel_spmd(nc, in_maps, core_ids=list(range(N_CORES)))
    out = np.concatenate([r["out16"] for r in res.results], axis=0)
    return out.astype(np.float32)


# revision 20
# speedup vs baseline: 1.5630x; 1.0256x over previous
"""AttentiveTransformer (linear -> ghost BN -> prior mask -> sparsemax) on 8 TRN2 cores.

v2: fp16 end-to-end. Layout: batch rows on partitions, G=2048 on free axis.
Each [128, 2048] tile is one ghost-BN chunk; 64 tiles per core.

Per tile c:
  fT   = DMA-transposed f (xbar), fp16                  [128f, b]x4k
  nm   = -colmean(f) via 4 tiny PE matmuls (negsel)     [128f, 4k] psum
  fcT  = fT + nm (DVE tensor_scalar, 4x mode)
  x    = fcT.T @ Wt (16 fp16 matmuls, f32 psum)
  xs   = copy(x) fp16 (Act)         xsq = xs*xs (DVE)
  var  = selector-matmul colmean(xsq) -> [8,512] psum shared by tile pair
  a    = sqrt(1/var): DVE reciprocal + Act sqrt -> fp16 row
  rbc  = row broadcast by 2-stage DMA (gather row -> stride-0 bcast)
  ma   = priors * rbc (GPSIMD)      z = xs * ma (DVE 2x)
  tau  = top8 (DVE max8) cumsum trick;  out = Relu(z + (-tau)) on Act -> fp16
"""
import numpy as np
from contextlib import ExitStack

import concourse.bass as bass
import concourse.bacc as bacc
import concourse.tile as tile
from concourse import mybir
from concourse.bass_utils import run_bass_kernel_spmd

F32 = mybir.dt.float32
F16 = mybir.dt.float16

B_FULL, IN, G = 65536, 512, 2048
N_CORES = 8
P = 128
KT = IN // P           # 4 k-tiles of 128
NT = G // 512          # 4 n-tiles of 512


def build(n_tiles, gamma_trivial):
    nc = bacc.Bacc()
    rows = n_tiles * P
    f16_d = nc.dram_tensor("f16", [rows, IN], F16, kind="ExternalInput")
    p16_d = nc.dram_tensor("p16", [rows, G], F16, kind="ExternalInput")
    wt_d = nc.dram_tensor("wt", [P, KT * G], F16, kind="ExternalInput")
    selbig_d = nc.dram_tensor("selbig", [P, 64], F16, kind="ExternalInput")
    negsel_d = nc.dram_tensor("negsel", [P, 16], F16, kind="ExternalInput")
    rinv8_d = nc.dram_tensor("rinv8", [P, 8], F32, kind="ExternalInput")
    grow8_d = nc.dram_tensor("grow8", [8, 512], F16, kind="ExternalInput")
    out_d = nc.dram_tensor("out16", [rows, G], F16, kind="ExternalOutput")

    with tile.TileContext(nc) as tc, ExitStack() as ctx:
        singles = ctx.enter_context(tc.tile_pool(name="singles", bufs=1))
        ftpool = ctx.enter_context(tc.tile_pool(name="ftpool", bufs=2))
        f4pool = ctx.enter_context(tc.tile_pool(name="f4pool", bufs=2))
        fcpool = ctx.enter_context(tc.tile_pool(name="fcpool", bufs=3))
        ppool = ctx.enter_context(tc.tile_pool(name="ppool", bufs=5))
        xspool = ctx.enter_context(tc.tile_pool(name="xspool", bufs=5))
        xqpool = ctx.enter_context(tc.tile_pool(name="xqpool", bufs=3))
        vrpool = ctx.enter_context(tc.tile_pool(name="vrpool", bufs=2))
        arpool = ctx.enter_context(tc.tile_pool(name="arpool", bufs=2))
        rbpool = ctx.enter_context(tc.tile_pool(name="rbpool", bufs=3))
        mapool = ctx.enter_context(tc.tile_pool(name="mapool", bufs=3))
        zpool = ctx.enter_context(tc.tile_pool(name="zpool", bufs=3))
        smpool = ctx.enter_context(tc.tile_pool(name="smpool", bufs=4))
        opool = ctx.enter_context(tc.tile_pool(name="opool", bufs=3))
        adram = ctx.enter_context(tc.tile_pool(name="adram", bufs=2, space="DRAM"))
        ps_px = ctx.enter_context(tc.tile_pool(name="ps_px", bufs=6, space="PSUM"))
        ps_v = ctx.enter_context(tc.tile_pool(name="ps_v", bufs=1, space="PSUM"))
        ps_nm = ctx.enter_context(tc.tile_pool(name="ps_nm", bufs=1, space="PSUM"))

        # ---- constants ----
        wt_t = singles.tile([P, KT, G], F16)
        nc.sync.dma_start(wt_t[:], wt_d[:].rearrange("p (k g) -> p k g", k=KT))
        selbig = singles.tile([P, 8, 8], F16)
        nc.sync.dma_start(selbig[:], selbig_d[:].rearrange("p (a b) -> p a b", a=8))
        negsel = singles.tile([P, KT, 4], F16)
        nc.sync.dma_start(negsel[:], negsel_d[:].rearrange("p (a b) -> p a b", a=KT))
        nrinv8 = singles.tile([P, 8], F32)
        nc.sync.dma_start(nrinv8[:], rinv8_d[:])
        zeros8 = singles.tile([P, 8], F32)
        nc.vector.memset(zeros8[:], 0.0)
        if not gamma_trivial:
            grow8 = singles.tile([8, 512], F16)
            nc.sync.dma_start(grow8[:], grow8_d[:])

        AL = mybir.AluOpType

        def front(c, fTg, f4, vps8, first_of_pair):
            t16, t4 = c % 16, c % 4
            p16t = ppool.tile([P, G], F16)
            nc.sync.dma_start(p16t[:], p16_d[c * P:(c + 1) * P, :])
            # nm[:, k] = -colmean over batch of f feature block k
            nm = ps_nm.tile([P, KT], F32)
            for k in range(KT):
                nc.tensor.matmul(nm[:], f4[:, t4, k * P:(k + 1) * P],
                                 negsel[:, k, :], start=(k == 0), stop=(k == KT - 1))
            fcT = fcpool.tile([P, KT, P], F16, tag="fcT")
            for k in range(KT):
                nc.vector.tensor_scalar(fcT[:, k, :],
                                        fTg[:, k, t16 * P:(t16 + 1) * P],
                                        nm[:, k:k + 1], None, op0=AL.add)
            pxq = [ps_px.tile([P, 512], F32, tag="px", name=f"px{n}")
                   for n in range(NT)]
            for k in range(KT):
                for n in range(NT):
                    nc.tensor.matmul(pxq[n][:], fcT[:, k, :],
                                     wt_t[:, k, n * 512:(n + 1) * 512],
                                     start=(k == 0), stop=(k == KT - 1))
            xs = xspool.tile([P, G], F16)
            for n in range(NT):
                nc.scalar.activation(xs[:, n * 512:(n + 1) * 512], pxq[n][:],
                                     mybir.ActivationFunctionType.Copy)
            xsq = xqpool.tile([P, G], F16)
            nc.vector.tensor_tensor(xsq[:], xs[:], xs[:], op=AL.mult)
            i = 0 if first_of_pair else 1
            for n in range(NT):
                nc.tensor.matmul(vps8[:], selbig[:, 4 * i + n, :],
                                 xsq[:, n * 512:(n + 1) * 512],
                                 start=(i == 0 and n == 0),
                                 stop=(i == 1 and n == NT - 1))
            return p16t, xs

        def finalize(vps8):
            vrec = vrpool.tile([8, 512], F32)
            nc.vector.reciprocal_approx_fast(vrec[:], vps8[:])
            arow = arpool.tile([8, 512], F16)
            nc.scalar.activation(arow[:], vrec[:], mybir.ActivationFunctionType.Sqrt)
            if not gamma_trivial:
                nc.vector.tensor_tensor(arow[:], arow[:], grow8[:], op=AL.mult)
            ascr = adram.tile([1, 4096], F16)
            nc.sync.dma_start(ascr[:], arow[:])
            rbc = rbpool.tile([P, 2, G], F16)
            nc.sync.dma_start(rbc[:].rearrange("p a b -> p (a b)"),
                              ascr[:].to_broadcast([P, 4096]))
            return rbc

        def back(c, i, p16t, xs, rbc):
            ma = mapool.tile([P, G], F16)
            nc.gpsimd.tensor_tensor(ma[:], p16t[:], rbc[:, i, :], op=AL.mult)
            z = zpool.tile([P, G], F16)
            nc.vector.tensor_tensor(z[:], xs[:], ma[:], op=AL.mult)
            m8 = smpool.tile([P, 8], F16, tag="m8")
            nc.vector.max(m8[:], z[:])
            cs = smpool.tile([P, 8], F32, tag="cs")
            nc.vector.tensor_tensor_scan(cs[:], m8[:], zeros8[:], 0.0,
                                         op0=AL.add, op1=AL.bypass)
            taur = smpool.tile([P, 8], F32, tag="taur")
            nc.vector.scalar_tensor_tensor(taur[:], in0=cs[:], scalar=-1.0,
                                           in1=nrinv8[:], op0=AL.add, op1=AL.mult)
            ntau = smpool.tile([P, 1], F32, tag="ntau")
            nc.vector.tensor_reduce(ntau[:], taur[:], axis=mybir.AxisListType.X,
                                    op=AL.min, negate=False)
            o16 = opool.tile([P, G], F16)
            nc.scalar.activation(o16[:], z[:], mybir.ActivationFunctionType.Relu,
                                 bias=ntau[:])
            nc.sync.dma_start(out_d[c * P:(c + 1) * P, :], o16[:])

        prev = None
        fTg = f4 = None
        for pr in range(n_tiles // 2):
            c0 = 2 * pr
            if c0 % 16 == 0:
                fTg = ftpool.tile([P, KT, 2048], F16)
                g0 = c0 * P
                for k in range(KT):
                    nc.sync.dma_start_transpose(
                        fTg[:, k, :], f16_d[g0:g0 + 2048, k * P:(k + 1) * P])
            if c0 % 4 == 0:
                f4 = f4pool.tile([P, 4, IN], F16)
                nc.sync.dma_start(
                    f4[:], f16_d[c0 * P:(c0 + 4) * P, :].rearrange(
                        "(t p) k -> p t k", p=P))
            vps8 = ps_v.tile([8, 512], F32)
            pA = front(c0, fTg, f4, vps8, True)
            pB = front(c0 + 1, fTg, f4, vps8, False)
            if prev is not None:
                (a0, a1), (xa, xb), rbc_p, cp = prev
                back(cp, 0, a0, xa, rbc_p)
                back(cp + 1, 1, a1, xb, rbc_p)
            rbc = finalize(vps8)
            prev = ((pA[0], pB[0]), (pA[1], pB[1]), rbc, c0)
        (a0, a1), (xa, xb), rbc_p, cp = prev
        back(cp, 0, a0, xa, rbc_p)
        back(cp + 1, 1, a1, xb, rbc_p)

    nc.finalize()
    return nc


_CACHE = {}


def _consts():
    selbig = np.zeros((P, 8, 8), np.float16)
    for i in range(8):
        selbig[:, i, i] = 1.0 / 128
    negsel = np.zeros((P, KT, 4), np.float16)
    for k in range(KT):
        negsel[:, k, k] = -1.0 / 128
    rinv8 = np.broadcast_to(-1.0 / np.arange(1, 9, dtype=np.float32), (P, 8)).copy()
    return selbig.reshape(P, 64), negsel.reshape(P, 16), rinv8


def kernel(priors, processed_feat, W, gamma, beta):
    feat = np.ascontiguousarray(processed_feat, dtype=np.float32)
    priors = np.ascontiguousarray(priors, dtype=np.float32)
    W = np.ascontiguousarray(W, dtype=np.float32)
    gamma = np.asarray(gamma, dtype=np.float32)
    beta = np.asarray(beta, dtype=np.float32)
    assert np.all(beta == 0.0), "beta != 0 path not implemented"
    gamma_trivial = bool(np.all(gamma == 1.0))

    B = feat.shape[0]
    shard = B // N_CORES
    n_tiles = shard // P

    f16 = feat.astype(np.float16)
    p16 = priors.astype(np.float16)
    # wt[p, k, g] = W[g, k*128+p]
    wt = np.ascontiguousarray(
        W.T.astype(np.float16).reshape(KT, P, G).transpose(1, 0, 2)
    ).reshape(P, KT * G)
    selbig, negsel, rinv8 = _consts()
    # grow8[4i+n, j] = gamma[n*512+j] for i in {0,1}
    gr = gamma.astype(np.float16).reshape(4, 512)
    grow8 = np.concatenate([gr, gr], axis=0)

    key = (n_tiles, gamma_trivial)
    if key not in _CACHE:
        _CACHE[key] = build(*key)
    nc = _CACHE[key]

    in_maps = []
    for i in range(N_CORES):
        in_maps.append({
            "f16": f16[i * shard:(i + 1) * shard],
            "p16": p16[i * shard:(i + 1) * shard],
            "wt": wt,
            "selbig": selbig,
            "negsel": negsel,
            "rinv8": rinv8,
            "grow8": grow8,
        })
    res = run_bass_kernel_spmd(nc, in_maps, core_ids=list(range(N_CORES)))
    out = np.concatenate([r["out16"] for r in res.results], axis=0)
    return out.astype(np.float32)


# revision 21
# speedup vs baseline: 1.7084x; 1.0931x over previous
"""AttentiveTransformer (linear -> ghost BN -> prior mask -> sparsemax) on 8 TRN2 cores.

v2: fp16 end-to-end. Layout: batch rows on partitions, G=2048 on free axis.
Each [128, 2048] tile is one ghost-BN chunk; 64 tiles per core.

Per tile c:
  fT   = DMA-transposed f (xbar), fp16                  [128f, b]x4k
  nm   = -colmean(f) via 4 tiny PE matmuls (negsel)     [128f, 4k] psum
  fcT  = fT + nm (DVE tensor_scalar, 4x mode)
  x    = fcT.T @ Wt (16 fp16 matmuls, f32 psum)
  xs   = copy(x) fp16 (Act)         xsq = xs*xs (DVE)
  var  = selector-matmul colmean(xsq) -> [8,512] psum shared by tile pair
  a    = sqrt(1/var): DVE reciprocal + Act sqrt -> fp16 row
  rbc  = row broadcast by 2-stage DMA (gather row -> stride-0 bcast)
  ma   = priors * rbc (GPSIMD)      z = xs * ma (DVE 2x)
  tau  = top8 (DVE max8) cumsum trick;  out = Relu(z + (-tau)) on Act -> fp16
"""
import numpy as np
from contextlib import ExitStack

import concourse.bass as bass
import concourse.bacc as bacc
import concourse.tile as tile
from concourse import mybir
from concourse.bass_utils import run_bass_kernel_spmd

F32 = mybir.dt.float32
F16 = mybir.dt.float16

B_FULL, IN, G = 65536, 512, 2048
N_CORES = 8
P = 128
KT = IN // P           # 4 k-tiles of 128
NT = G // 512          # 4 n-tiles of 512


def build(n_tiles, gamma_trivial):
    nc = bacc.Bacc()
    rows = n_tiles * P
    f16_d = nc.dram_tensor("f16", [rows, IN], F16, kind="ExternalInput")
    p16_d = nc.dram_tensor("p16", [rows, G], F16, kind="ExternalInput")
    wt_d = nc.dram_tensor("wt", [P, KT * G], F16, kind="ExternalInput")
    selbig_d = nc.dram_tensor("selbig", [P, 64], F16, kind="ExternalInput")
    negsel_d = nc.dram_tensor("negsel", [P, 16], F16, kind="ExternalInput")
    rinv8_d = nc.dram_tensor("rinv8", [P, 8], F32, kind="ExternalInput")
    grow8_d = nc.dram_tensor("grow8", [8, 512], F16, kind="ExternalInput")
    out_d = nc.dram_tensor("out16", [rows, G], F16, kind="ExternalOutput")

    with tile.TileContext(nc) as tc, ExitStack() as ctx:
        singles = ctx.enter_context(tc.tile_pool(name="singles", bufs=1))
        ftpool = ctx.enter_context(tc.tile_pool(name="ftpool", bufs=2))
        f4pool = ctx.enter_context(tc.tile_pool(name="f4pool", bufs=2))
        fcpool = ctx.enter_context(tc.tile_pool(name="fcpool", bufs=3))
        ppool = ctx.enter_context(tc.tile_pool(name="ppool", bufs=5))
        xspool = ctx.enter_context(tc.tile_pool(name="xspool", bufs=5))
        xqpool = ctx.enter_context(tc.tile_pool(name="xqpool", bufs=3))
        vrpool = ctx.enter_context(tc.tile_pool(name="vrpool", bufs=2))
        arpool = ctx.enter_context(tc.tile_pool(name="arpool", bufs=2))
        rbpool = ctx.enter_context(tc.tile_pool(name="rbpool", bufs=3))
        mapool = ctx.enter_context(tc.tile_pool(name="mapool", bufs=3))
        zpool = ctx.enter_context(tc.tile_pool(name="zpool", bufs=3))
        smpool = ctx.enter_context(tc.tile_pool(name="smpool", bufs=4))
        opool = ctx.enter_context(tc.tile_pool(name="opool", bufs=3))
        adram = ctx.enter_context(tc.tile_pool(name="adram", bufs=2, space="DRAM"))
        ps_px = ctx.enter_context(tc.tile_pool(name="ps_px", bufs=6, space="PSUM"))
        ps_v = ctx.enter_context(tc.tile_pool(name="ps_v", bufs=1, space="PSUM"))
        ps_nm = ctx.enter_context(tc.tile_pool(name="ps_nm", bufs=1, space="PSUM"))

        # ---- constants ----
        wt_t = singles.tile([P, KT, G], F16)
        nc.sync.dma_start(wt_t[:], wt_d[:].rearrange("p (k g) -> p k g", k=KT))
        selbig = singles.tile([P, 8, 8], F16)
        nc.sync.dma_start(selbig[:], selbig_d[:].rearrange("p (a b) -> p a b", a=8))
        negsel = singles.tile([P, KT, 4], F16)
        nc.sync.dma_start(negsel[:], negsel_d[:].rearrange("p (a b) -> p a b", a=KT))
        nrinv8 = singles.tile([P, 8], F32)
        nc.sync.dma_start(nrinv8[:], rinv8_d[:])
        zeros8 = singles.tile([P, 8], F32)
        nc.vector.memset(zeros8[:], 0.0)
        if not gamma_trivial:
            grow8 = singles.tile([8, 512], F16)
            nc.sync.dma_start(grow8[:], grow8_d[:])

        AL = mybir.AluOpType

        def front(c, fTg, f4, vps8, first_of_pair):
            t16, t4 = c % 16, c % 4
            p16t = ppool.tile([P, G], F16)
            nc.sync.dma_start(p16t[:], p16_d[c * P:(c + 1) * P, :])
            # nm[:, k] = -colmean over batch of f feature block k
            nm = ps_nm.tile([P, KT], F32)
            for k in range(KT):
                nc.tensor.matmul(nm[:], f4[:, t4, k * P:(k + 1) * P],
                                 negsel[:, k, :], start=(k == 0), stop=(k == KT - 1))
            nmsb = fcpool.tile([P, KT], F32, tag="nmsb")
            nc.scalar.activation(nmsb[:], nm[:], mybir.ActivationFunctionType.Copy)
            fcT = fcpool.tile([P, KT, P], F16, tag="fcT")
            for k in range(KT):
                nc.vector.tensor_scalar(fcT[:, k, :],
                                        fTg[:, k, t16 * P:(t16 + 1) * P],
                                        nmsb[:, k:k + 1], None, op0=AL.add)
            pxq = [ps_px.tile([P, 512], F32, tag="px", name=f"px{n}")
                   for n in range(NT)]
            for k in range(KT):
                for n in range(NT):
                    nc.tensor.matmul(pxq[n][:], fcT[:, k, :],
                                     wt_t[:, k, n * 512:(n + 1) * 512],
                                     start=(k == 0), stop=(k == KT - 1))
            xs = xspool.tile([P, G], F16)
            for n in range(NT):
                nc.scalar.activation(xs[:, n * 512:(n + 1) * 512], pxq[n][:],
                                     mybir.ActivationFunctionType.Copy)
            xsq = xqpool.tile([P, G], F16)
            nc.vector.tensor_tensor(xsq[:], xs[:], xs[:], op=AL.mult)
            i = 0 if first_of_pair else 1
            for n in range(NT):
                nc.tensor.matmul(vps8[:], selbig[:, 4 * i + n, :],
                                 xsq[:, n * 512:(n + 1) * 512],
                                 start=(i == 0 and n == 0),
                                 stop=(i == 1 and n == NT - 1))
            return p16t, xs

        def finalize(vps8):
            vrec = vrpool.tile([8, 512], F32)
            nc.vector.reciprocal_approx_fast(vrec[:], vps8[:])
            arow = arpool.tile([8, 512], F16)
            nc.scalar.activation(arow[:], vrec[:], mybir.ActivationFunctionType.Sqrt)
            if not gamma_trivial:
                nc.vector.tensor_tensor(arow[:], arow[:], grow8[:], op=AL.mult)
            ascr = adram.tile([1, 4096], F16)
            nc.sync.dma_start(ascr[:], arow[:])
            rbc = rbpool.tile([P, 2, G], F16)
            nc.sync.dma_start(rbc[:].rearrange("p a b -> p (a b)"),
                              ascr[:].to_broadcast([P, 4096]))
            return rbc

        def back(c, i, p16t, xs, rbc):
            ma = mapool.tile([P, G], F16)
            nc.gpsimd.tensor_tensor(ma[:], p16t[:], rbc[:, i, :], op=AL.mult)
            z = zpool.tile([P, G], F16)
            nc.vector.tensor_tensor(z[:], xs[:], ma[:], op=AL.mult)
            m8 = smpool.tile([P, 8], F16, tag="m8")
            nc.vector.max(m8[:], z[:])
            cs = smpool.tile([P, 8], F32, tag="cs")
            nc.vector.tensor_tensor_scan(cs[:], m8[:], zeros8[:], 0.0,
                                         op0=AL.add, op1=AL.bypass)
            taur = smpool.tile([P, 8], F32, tag="taur")
            nc.vector.scalar_tensor_tensor(taur[:], in0=cs[:], scalar=-1.0,
                                           in1=nrinv8[:], op0=AL.add, op1=AL.mult)
            ntau = smpool.tile([P, 1], F32, tag="ntau")
            nc.vector.tensor_reduce(ntau[:], taur[:], axis=mybir.AxisListType.X,
                                    op=AL.min, negate=False)
            o16 = opool.tile([P, G], F16)
            nc.scalar.activation(o16[:], z[:], mybir.ActivationFunctionType.Relu,
                                 bias=ntau[:])
            nc.sync.dma_start(out_d[c * P:(c + 1) * P, :], o16[:])

        prev = None
        fTg = f4 = None
        for pr in range(n_tiles // 2):
            c0 = 2 * pr
            if c0 % 16 == 0:
                fTg = ftpool.tile([P, KT, 2048], F16)
                g0 = c0 * P
                for k in range(KT):
                    nc.sync.dma_start_transpose(
                        fTg[:, k, :], f16_d[g0:g0 + 2048, k * P:(k + 1) * P])
            if c0 % 4 == 0:
                f4 = f4pool.tile([P, 4, IN], F16)
                nc.sync.dma_start(
                    f4[:], f16_d[c0 * P:(c0 + 4) * P, :].rearrange(
                        "(t p) k -> p t k", p=P))
            vps8 = ps_v.tile([8, 512], F32)
            pA = front(c0, fTg, f4, vps8, True)
            pB = front(c0 + 1, fTg, f4, vps8, False)
            if prev is not None:
                (a0, a1), (xa, xb), rbc_p, cp = prev
                back(cp, 0, a0, xa, rbc_p)
                back(cp + 1, 1, a1, xb, rbc_p)
            rbc = finalize(vps8)
            prev = ((pA[0], pB[0]), (pA[1], pB[1]), rbc, c0)
        (a0, a1), (xa, xb), rbc_p, cp = prev
        back(cp, 0, a0, xa, rbc_p)
        back(cp + 1, 1, a1, xb, rbc_p)

    nc.finalize()
    return nc


_CACHE = {}


def _consts():
    selbig = np.zeros((P, 8, 8), np.float16)
    for i in range(8):
        selbig[:, i, i] = 1.0 / 128
    negsel = np.zeros((P, KT, 4), np.float16)
    for k in range(KT):
        negsel[:, k, k] = -1.0 / 128
    rinv8 = np.broadcast_to(-1.0 / np.arange(1, 9, dtype=np.float32), (P, 8)).copy()
    return selbig.reshape(P, 64), negsel.reshape(P, 16), rinv8


def kernel(priors, processed_feat, W, gamma, beta):
    feat = np.ascontiguousarray(processed_feat, dtype=np.float32)
    priors = np.ascontiguousarray(priors, dtype=np.float32)
    W = np.ascontiguousarray(W, dtype=np.float32)
    gamma = np.asarray(gamma, dtype=np.float32)
    beta = np.asarray(beta, dtype=np.float32)
    assert np.all(beta == 0.0), "beta != 0 path not implemented"
    gamma_trivial = bool(np.all(gamma == 1.0))

    B = feat.shape[0]
    shard = B // N_CORES
    n_tiles = shard // P

    f16 = feat.astype(np.float16)
    p16 = priors.astype(np.float16)
    # wt[p, k, g] = W[g, k*128+p]
    wt = np.ascontiguousarray(
        W.T.astype(np.float16).reshape(KT, P, G).transpose(1, 0, 2)
    ).reshape(P, KT * G)
    selbig, negsel, rinv8 = _consts()
    # grow8[4i+n, j] = gamma[n*512+j] for i in {0,1}
    gr = gamma.astype(np.float16).reshape(4, 512)
    grow8 = np.concatenate([gr, gr], axis=0)

    key = (n_tiles, gamma_trivial)
    if key not in _CACHE:
        _CACHE[key] = build(*key)
    nc = _CACHE[key]

    in_maps = []
    for i in range(N_CORES):
        in_maps.append({
            "f16": f16[i * shard:(i + 1) * shard],
            "p16": p16[i * shard:(i + 1) * shard],
            "wt": wt,
            "selbig": selbig,
            "negsel": negsel,
            "rinv8": rinv8,
            "grow8": grow8,
        })
    res = run_bass_kernel_spmd(nc, in_maps, core_ids=list(range(N_CORES)))
    out = np.concatenate([r["out16"] for r in res.results], axis=0)
    return out.astype(np.float32)
